# revision 28
# baseline (speedup 1.0000x reference)
"""Causal single-head attention on 8 Trainium2 NeuronCores (Bass/Tile).

Problem: x [4, 2048, 1024], W_{q,k,v} [1024, 1024] (torch Linear layout,
y = x @ W.T), causal softmax(QK^T/sqrt(D)) @ V  ->  [4, 2048, 1024] fp32.

Sharding (uniform SPMD program, per-core data only):
  core c -> batch b = c//2, key-parity h = c%2.
  Each core computes attention for ALL 2048 queries of its batch against
  the 1024 keys with original index = h (mod 2) ("virtual" keys k' with
  global key = 2k' + h), flash-style transposed (S^T[k', q] tiles),
  unnormalized: O_part = sum_k exp(s) V, l_part = sum_k exp(s). Host
  combines: out[b] = (O_0 + O_1) / (l_0 + l_1). Causality over virtual
  keys makes every (k'-tile j, q-chunk i) block with j < i fully allowed
  and the j == i block maskable with one slot-independent pattern
  (allowed iff q_l >= 2*k_l + h), so all 8 core programs are IDENTICAL.

  Wq/Wk folded on host: scores = x_q @ (Wq^T Wk) @ x_k^T, so the device
  does 2 projections (C = x_k G, V = x_k Wv^T), not 3.

Precision: everything bf16 on the PE (1 cyc/row), fp32 PSUM
accumulation, bf16 output drains (host renormalizes in fp32). fp8 was
re-evaluated this session: DoubleRow measures exactly 2.0x bf16 on HW
(883ns vs 1763ns per K=1024 [128,512] chain), so 3-term residual-split
fp8 (the only variant that passes the 2e-2 gate; naive fp8 = 4-6e-2)
costs 1.5x bf16 -> fp8 rejected for good.

Schedule: single flat pool scope (no inter-phase PSUM pool barriers).
V projection first (first matmul needs only ~0.5 MB of DMA), then C,
query-side x^T streamed behind. A 256-wide warm-up matmul chain burns
the initial DMA wait and hands off to the first real chain with no
idle gap, so real chains start at the full 2.4GHz p-state (an idle gap
drops the PE clock to ~1.2GHz for ~3us). Three S units are emitted
between the two c_proj halves to bridge the projection->attention
transition. Attention is one global software pipeline: S units run
LOOK ahead of the AV consumers; scalar does exp, vector drains, gpsimd
masks. Slots drain in dv-halves; the last slot's final half drains in
256-col quarters on scalar to shorten the tail.
"""

import numpy as np
import ml_dtypes

import concourse.mybir as mybir
import concourse.tile as tile
from concourse import bacc
from concourse.bass_utils import run_bass_kernel_spmd

F32 = mybir.dt.float32
BF = mybir.dt.bfloat16
BF_NP = ml_dtypes.bfloat16

B, S, D = 4, 2048, 1024
NP = 128  # partitions
DP = D // NP  # 8 contraction-dim tiles
ET = D // NP  # 8 output-dim tiles
KP = S // 2  # 1024 keys per core
KT = KP // NP  # 8 key tiles
QCH = 256  # per-slot query width
QW = 2 * QCH  # paired-slot width
NSLOT = S // QCH  # 8 slots
NPAIR = NSLOT // 2  # 4 slot pairs
SCALE = 1.0 / 32.0  # 1/sqrt(D)
EBIAS = -1.5  # exp bias (cancels in O/l; numerical headroom)
LOOK = 4  # S-unit runahead, limited by 5 shared psum ring bufs
NWARM = 20

_NC_CACHE = {}


def _build_nc():
    nc = bacc.Bacc(None, target_bir_lowering=False)

    # host-pretiled inputs, contiguous per partition for fat few-issue DMAs
    xt = nc.dram_tensor("xt", [NP, NPAIR, DP, QW], BF, kind="ExternalInput")
    xka = nc.dram_tensor("xka", [4, NP, DP, NP], BF, kind="ExternalInput")
    xkb = nc.dram_tensor("xkb", [NP, DP, 512], BF, kind="ExternalInput")
    # wgt = Wk^T @ Wq (host-folded QK^T kernel matrix): [p, dp, e]
    wgt = nc.dram_tensor("wgt", [NP, DP, D], BF, kind="ExternalInput")
    # wvt quarter 0 split in 128-col eighths (fine first-chain DMA), rest
    # of Wv^T as 256-col quarters 1-3
    wvt_a = nc.dram_tensor("wvt_a", [2, NP, DP, 128], BF, kind="ExternalInput")
    wvt_b = nc.dram_tensor("wvt_b", [3, NP, DP, 256], BF, kind="ExternalInput")
    # 0/1 causal mask: left half = diag pattern, right half = ones
    mask = nc.dram_tensor("mask", [NP, QW], BF, kind="ExternalInput")
    ones = nc.dram_tensor("ones", [NP, 2], BF, kind="ExternalInput")
    o_out = nc.dram_tensor("o", [S, D], BF, kind="ExternalOutput")
    l_out = nc.dram_tensor("l", [2, NSLOT, QCH], BF, kind="ExternalOutput")

    o_r = o_out.rearrange("(t p) d -> p t d", p=NP)  # [128, 16, 1024]

    sunits = [(pi, j) for pi in range(NPAIR) for j in range(2 * pi + 2)]
    soff = [0, 2, 6, 12]  # global index of (pi, 0)
    NS = len(sunits)

    with tile.TileContext(nc) as tc:
        with (
            tc.tile_pool(name="res", bufs=1) as res,
            tc.tile_pool(name="wp", bufs=1) as wp,
            tc.tile_pool(name="xp", bufs=1) as xp,
            tc.tile_pool(name="pbp", bufs=8) as pbp,
            tc.tile_pool(name="pop", bufs=2) as pop,
            tc.tile_pool(name="prp", bufs=2) as prp,
            tc.tile_pool(name="ost", bufs=2) as ost,
            tc.tile_pool(name="psA", bufs=5, space="PSUM") as psA,
            tc.tile_pool(name="ops", bufs=1, space="PSUM") as ops,
            tc.tile_pool(name="lps", bufs=1, space="PSUM") as lps,
        ):
            xt_res = res.tile([NP, NPAIR, DP, QW], BF)  # 16KB/p
            # ct split by key slab: tile-granular deps mean S units for
            # keys 0-511 must not wait on the last c_proj(1) copy
            ct_res = [
                res.tile([NP, ET, KP // 2], BF, name=f"ct{i}")
                for i in range(2)
            ]
            v_res = res.tile([NP, KT, D], BF)  # 16KB/p
            t_mask = res.tile([NP, QW], BF)
            t_ones = res.tile([NP, 2], BF)
            t_bias = res.tile([NP, 1], F32)
            l_acc = res.tile([2, NSLOT, QCH], BF)
            nc.gpsimd.memset(t_bias[:], EBIAS)

            wv_sb = wp.tile([NP, 4, DP, 256], BF, tag="wv", name="wv")
            wg_sb = wp.tile([NP, DP, D], BF, tag="wg", name="wg")
            xk_sb = [
                xp.tile([NP, DP, 512], BF, tag=f"xk{s_}", name=f"xk{s_}")
                for s_ in range(2)
            ]

            # DMA issue order = urgency order. All inputs stay on ONE
            # queue: a second queue steals bandwidth from the critical
            # first-chain pieces.
            nc.sync.dma_start(wv_sb[:, 0, :, 0:128], wvt_a[0])
            nc.sync.dma_start(xk_sb[0][:, :, 0:NP], xka[0])
            nc.sync.dma_start(wv_sb[:, 0, :, 128:256], wvt_a[1])
            nc.sync.dma_start(xk_sb[0][:, :, NP : 2 * NP], xka[1])
            nc.sync.dma_start(wv_sb[:, 1], wvt_b[0])
            for sub in range(2, 4):
                nc.sync.dma_start(
                    xk_sb[0][:, :, sub * NP : (sub + 1) * NP], xka[sub]
                )
            nc.sync.dma_start(wv_sb[:, 2], wvt_b[1])
            nc.sync.dma_start(wv_sb[:, 3], wvt_b[2])
            nc.sync.dma_start(wg_sb[:], wgt[:])
            nc.sync.dma_start(xk_sb[1][:], xkb[:])
            nc.sync.dma_start(t_mask[:], mask[:])
            nc.sync.dma_start(t_ones[:], ones[:])
            nc.sync.dma_start(xt_res[:], xt[:])

            # PE p-state warm-up (see module docstring)
            warm = wp.tile([NP, 512], BF, tag="warm", name="warm")
            nc.gpsimd.memset(warm[:], 0.25)
            wps = psA.tile([NP, 512], F32, tag="ps", name="warmps")
            for r in range(NWARM):
                nc.tensor.matmul(
                    wps[:, 0:256], warm[:, 0:NP], warm[:, 0:256],
                    start=(r == 0), stop=(r == NWARM - 1),
                )

            def v_proj256(kt_i, q4):
                xc = xk_sb[kt_i // 4]
                sub = kt_i % 4
                ps = psA.tile([NP, 512], F32, tag="ps", name=f"psv{kt_i}_{q4}")
                for dp in range(DP):
                    nc.tensor.matmul(
                        ps[:, 0:256],
                        xc[:, dp, sub * NP : (sub + 1) * NP],
                        wv_sb[:, q4, dp, :],
                        start=(dp == 0),
                        stop=(dp == DP - 1),
                    )
                nc.vector.tensor_copy(
                    v_res[:, kt_i, q4 * 256 : (q4 + 1) * 256], ps[:, 0:256]
                )

            def v_proj128(kt_i, s8):
                # 128-wide first units: only a 256KB wv slab + one xka
                # slab must have landed -> earliest possible PE start
                xc = xk_sb[0]
                ps = psA.tile([NP, 512], F32, tag="ps", name=f"psh{kt_i}_{s8}")
                for dp in range(DP):
                    nc.tensor.matmul(
                        ps[:, 0:128],
                        xc[:, dp, kt_i * NP : (kt_i + 1) * NP],
                        wv_sb[:, 0, dp, s8 * 128 : (s8 + 1) * 128],
                        start=(dp == 0),
                        stop=(dp == DP - 1),
                    )
                nc.vector.tensor_copy(
                    v_res[:, kt_i, s8 * 128 : (s8 + 1) * 128], ps[:, 0:128]
                )

            def v_proj512(kt_i, dv):
                xc = xk_sb[kt_i // 4]
                sub = kt_i % 4
                ps = psA.tile([NP, 512], F32, tag="ps", name=f"psw{kt_i}_{dv}")
                for dp in range(DP):
                    nc.tensor.matmul(
                        ps[:],
                        xc[:, dp, sub * NP : (sub + 1) * NP],
                        wv_sb[:, 2 * dv : 2 * dv + 2, dp, :],
                        start=(dp == 0),
                        stop=(dp == DP - 1),
                    )
                nc.vector.tensor_copy(
                    v_res[:, kt_i, dv * 512 : (dv + 1) * 512], ps[:]
                )

            def c_proj(ks, et):
                ps = psA.tile([NP, 512], F32, tag="ps", name=f"psk{ks}_{et}")
                for dp in range(DP):
                    nc.tensor.matmul(
                        ps[:],
                        wg_sb[:, dp, et * NP : (et + 1) * NP],
                        xk_sb[ks][:, dp, :],
                        start=(dp == 0),
                        stop=(dp == DP - 1),
                    )
                nc.vector.tensor_copy(ct_res[ks][:, et, :], ps[:])

            # ---------------- attention units ----------------
            # S production units per slot-pair pi (slots 2pi, 2pi+1):
            #   j <= 2pi     : paired-slot [128, QW] (j == 2pi is diag for
            #                  slot 2pi via mask; fully allowed for 2pi+1)
            #   j == 2pi + 1 : single-slot [128, QCH] (diag for slot 2pi+1)
            pb_t = {}
            pbo_t = {}
            l_ps = {}

            def s_unit(k):
                pi, j = sunits[k]
                s_ps = psA.tile([NP, QW], F32, tag="ps", name=f"s{pi}_{j}")
                if j == 2 * pi + 1:  # odd diag: single slot
                    for et in range(ET):
                        nc.tensor.matmul(
                            s_ps[:, 0:QCH],
                            ct_res[j // 4][:, et, (j % 4) * NP : (j % 4 + 1) * NP],
                            xt_res[:, pi, et, QCH:QW],
                            start=(et == 0),
                            stop=(et == ET - 1),
                        )
                    praw = prp.tile([NP, QCH], BF, tag="pro", name=f"pro{pi}")
                    nc.scalar.activation(
                        out=praw[:],
                        in_=s_ps[:, 0:QCH],
                        func=mybir.ActivationFunctionType.Exp,
                        scale=SCALE,
                        bias=t_bias[:],
                    )
                    p_t = pop.tile([NP, QCH], BF, tag="pbo", name=f"pbo{pi}")
                    nc.gpsimd.tensor_mul(p_t[:], praw[:], t_mask[:, 0:QCH])
                    pbo_t[pi] = p_t
                else:  # paired slot [128, QW]
                    for et in range(ET):
                        nc.tensor.matmul(
                            s_ps[:],
                            ct_res[j // 4][:, et, (j % 4) * NP : (j % 4 + 1) * NP],
                            xt_res[:, pi, et, :],
                            start=(et == 0),
                            stop=(et == ET - 1),
                        )
                    p_t = pbp.tile([NP, QW], BF, tag="pb", name=f"pb{pi}_{j}")
                    if j == 2 * pi:  # diag for slot 2pi: mask left half
                        praw = prp.tile([NP, QW], BF, tag="pre", name=f"pre{pi}")
                        nc.scalar.activation(
                            out=praw[:],
                            in_=s_ps[:],
                            func=mybir.ActivationFunctionType.Exp,
                            scale=SCALE,
                            bias=t_bias[:],
                        )
                        nc.gpsimd.tensor_mul(p_t[:], praw[:], t_mask[:])
                    else:
                        nc.scalar.activation(
                            out=p_t[:],
                            in_=s_ps[:],
                            func=mybir.ActivationFunctionType.Exp,
                            scale=SCALE,
                            bias=t_bias[:],
                        )
                    pb_t[(pi, j)] = p_t

            sp = 0

            def ensure(need_idx):
                nonlocal sp
                target = min(need_idx + 1 + LOOK, NS)
                while sp < target:
                    s_unit(sp)
                    sp += 1

            # ---------------- projections ----------------
            v_proj128(0, 0)
            v_proj128(0, 1)
            v_proj128(1, 0)
            v_proj128(1, 1)
            v_proj256(0, 1)
            v_proj256(1, 1)
            for kt_i in range(2, 4):
                v_proj512(kt_i, 0)
            for kt_i in range(4):
                v_proj512(kt_i, 1)
            for et in range(ET):
                c_proj(0, et)
            for kt_i in range(4, 8):
                v_proj512(kt_i, 0)
                v_proj512(kt_i, 1)
            for et in range(ET):
                c_proj(1, et)

            # ---------------- attention slot loop ----------------
            def drain_half(sl, dv, o_cur, do_l):
                # bf16 drains: halves PSUM-copy write traffic and the
                # output DMA (tail-critical); host combines in fp32.
                # q0 on vector, q1 on scalar: halves the copy latency the
                # next dv-pass (reusing these PSUM banks) must wait for.
                # One fused DMA per half-slot keeps sync-issue count low.
                if do_l:
                    nc.vector.tensor_copy(l_acc[:, sl, :], l_ps[sl][:])
                if sl == NSLOT - 1 and dv == 1:
                    # final piece: small quarters so the last copy+DMA
                    # after the last matmul is short
                    ot = ost.tile([NP, 2, 512], BF, tag="otd", name="ot_tail")
                    nc.vector.tensor_copy(ot[:, 0], o_cur[0][:])
                    nc.sync.dma_start(
                        o_r[:, sl * 2, dv * 512 : (dv + 1) * 512], ot[:, 0]
                    )
                    for hq in range(2):
                        cs = slice(hq * 256, (hq + 1) * 256)
                        nc.scalar.activation(
                            out=ot[:, 1, cs], in_=o_cur[1][:, cs],
                            func=mybir.ActivationFunctionType.Copy,
                        )
                        nc.scalar.dma_start(
                            o_r[:, sl * 2 + 1,
                                dv * 512 + hq * 256 : dv * 512 + (hq + 1) * 256],
                            ot[:, 1, cs],
                        )
                    return
                ot = ost.tile([NP, 2, 512], BF, tag="otd", name=f"ot{sl}_{dv}")
                nc.vector.tensor_copy(ot[:, 0], o_cur[0][:])
                nc.scalar.activation(
                    out=ot[:, 1], in_=o_cur[1][:],
                    func=mybir.ActivationFunctionType.Copy,
                )
                nc.sync.dma_start(
                    o_r[:, sl * 2 : sl * 2 + 2, dv * 512 : (dv + 1) * 512],
                    ot[:],
                )

            # Every slot accumulates O in two dv-half passes over its j
            # blocks: halves O's PSUM footprint (2 banks, enabling the
            # 5-deep score-bank runahead) and overlaps each dv0 drain
            # with the dv1 matmuls. Pass dv1 prefetches the NEXT slot's
            # S units between its AV blocks.
            for sl in range(NSLOT):
                pi, inp = sl // 2, sl % 2
                qo = inp * QCH
                soff_next = soff[(sl + 1) // 2] if sl + 1 < NSLOT else NS - 1
                l_ps[sl] = lps.tile([2, QCH], F32, tag="l", name=f"l{sl}")
                for dv in range(2):
                    o_cur = [
                        ops.tile(
                            [NP, 512], F32, tag=f"o{q}", name=f"o{sl}_{dv}_{q}"
                        )
                        for q in range(2)
                    ]
                    for j in range(sl + 1):
                        if dv == 0:
                            ensure(soff[pi] + j)
                        else:
                            ensure(min(soff_next + j, NS - 1))
                        first, last = (j == 0), (j == sl)
                        if inp == 1 and j == sl:
                            pt, coff = pbo_t[pi], 0
                        else:
                            pt, coff = pb_t[(pi, j)], qo
                        if dv == 0:
                            nc.tensor.matmul(
                                l_ps[sl][:],
                                t_ones[:],
                                pt[:, coff : coff + QCH],
                                start=first,
                                stop=last,
                            )
                        for q in range(2):
                            nc.tensor.matmul(
                                o_cur[q][:],
                                pt[:, coff + q * NP : coff + (q + 1) * NP],
                                v_res[:, j, dv * 512 : (dv + 1) * 512],
                                start=first,
                                stop=last,
                            )
                    drain_half(sl, dv, o_cur, do_l=(dv == 0))
            nc.sync.dma_start(l_out[:], l_acc[:])
    nc.compile()
    return nc


def _get_nc():
    if "nc" not in _NC_CACHE:
        _NC_CACHE["nc"] = _build_nc()
    return _NC_CACHE["nc"]


def kernel(x, W_query, W_key, W_value):
    x = np.asarray(x, dtype=np.float32)
    # fold Wq/Wk: scores = x_q @ (Wq^T Wk) @ x_k^T; device computes
    # C^T[e, k'] with stationary wgt[d, e] = (Wk^T @ Wq)[d, e]
    G = (
        np.asarray(W_key, dtype=np.float64).T @ np.asarray(W_query, dtype=np.float64)
    ).astype(BF_NP)
    wgt_a = np.ascontiguousarray(G.reshape(DP, NP, D).transpose(1, 0, 2))
    wvt_f = np.asarray(W_value, dtype=np.float32).T.astype(BF_NP)  # [D, D]
    wvt_a = np.ascontiguousarray(
        wvt_f.reshape(DP, NP, 8, 128).transpose(2, 1, 0, 3)[0:2]
    )
    wvt_b = np.ascontiguousarray(
        wvt_f.reshape(DP, NP, 4, 256).transpose(2, 1, 0, 3)[1:4]
    )

    ones_a = np.ones((NP, 2), dtype=BF_NP)
    k_l = np.arange(NP)[:, None]
    q_l = np.arange(QCH)[None, :]

    in_maps = []
    for c in range(8):
        b, h = c // 2, c % 2
        xb = x[b]
        # queries bf16: xt[p, pi, dp, qw] = x[b, pi*512+qw, dp*128+p]
        xt_t = np.ascontiguousarray(
            xb.reshape(NPAIR, QW, DP, NP).transpose(3, 0, 2, 1).astype(BF_NP)
        )
        # keys (parity h): fine slabs for keys 0-511, coarse for 512-1023
        xkv = xb[h::2].astype(BF_NP)  # [KP, D]
        xka_t = np.ascontiguousarray(
            xkv[:512].reshape(4, NP, DP, NP).transpose(0, 3, 2, 1)
        )
        xkb_t = np.ascontiguousarray(
            xkv[512:].reshape(512, DP, NP).transpose(2, 1, 0)
        )
        mask_a = np.ones((NP, QW), dtype=BF_NP)
        mask_a[:, 0:QCH] = (q_l >= 2 * k_l + h).astype(BF_NP)
        in_maps.append(
            {
                "xt": xt_t,
                "xka": xka_t,
                "xkb": xkb_t,
                "wgt": wgt_a,
                "wvt_a": wvt_a,
                "wvt_b": wvt_b,
                "mask": mask_a,
                "ones": ones_a,
            }
        )

    nc = _get_nc()
    res = run_bass_kernel_spmd(nc, in_maps, core_ids=list(range(8)))
    _NC_CACHE["last_res"] = res
    if res.exec_time_ns is not None:
        print(f"HW exec time: {res.exec_time_ns} ns")

    out = np.empty((B, S, D), dtype=np.float32)
    for b in range(B):
        o0 = np.asarray(res.results[2 * b]["o"], dtype=np.float32)
        o1 = np.asarray(res.results[2 * b + 1]["o"], dtype=np.float32)
        l0 = np.asarray(
            res.results[2 * b]["l"][0], dtype=np.float32
        ).reshape(S, 1)
        l1 = np.asarray(
            res.results[2 * b + 1]["l"][0], dtype=np.float32
        ).reshape(S, 1)
        out[b] = (o0 + o1) / (l0 + l1)
    return out
